# revision 4
# baseline (speedup 1.0000x reference)
"""Trainium2 Bass kernel for nn_GCN_19791209300130 (hypergraph GCN, 8 cores).

Strategy: densify the sparse hypergraph incidence structure into a count
matrix C [E, N] built on host from edge_index (index-only preprocessing).
With LC = ln(C) (-60 for zeros), the attention-weighted scatter phases become
dense matmuls:
    CEX[e,n]   = exp(lrelu_0.2(ax[n] + ae[e]) + LC[e,n])   (= C * exp(leaky(a)))
    denom[e]   = sum_n CEX[e,n] + 1e-16
    m'[e,:]    = (B[e]/denom[e]^2) * (CEX @ xw)[e,:]
    convT[f,n] = D[n] * sum_e m'[e,f]*CEX[e,n] + bias[f]
This matches PyG HypergraphConv exactly (softmax is shift-invariant so the
segment-max shift can be dropped; logits are ~N(0,0.6), no overflow risk).

Sharding (8 cores): core k owns node slice Nk and edge slice Ek (512 each).
Per conv: xw sharded by n + AllGather; m-phase sharded by e (contract all n);
m' AllGather; out-phase sharded by n (contract all e). Attention head sharded
by att1-row j with AllReduce of the s partials. GraphNorm via AllReduced raw
moments. Weights replicated.
"""
import numpy as np
import ml_dtypes

import concourse.bass as bass
import concourse.bacc as bacc
import concourse.tile as tile
from concourse import mybir
from concourse.bass_utils import run_bass_kernel_spmd

NCORES = 8
N = 4096
E = 4096
F = 1024
HID = 512
S = N // NCORES      # 512 shard
NT = S // 128        # 4
KT = F // 128        # 8
NK = N // 128        # 32

F32 = mybir.dt.float32
BF16 = mybir.dt.bfloat16
AF = mybir.ActivationFunctionType
ALU = mybir.AluOpType
AX = mybir.AxisListType.X

_CACHE = {}


def _bcast(t, offset, step, count, parts=128):
    """DRAM AP broadcast across partitions: count elems at offset with step."""
    return bass.AP(tensor=t.ap().tensor, offset=offset,
                   ap=[[0, parts], [step, count]])


def build_program():
    nc = bacc.Bacc("TRN2", target_bir_lowering=False, debug=False,
                   num_devices=NCORES)

    # ---------------- inputs ----------------
    t_xT = nc.dram_tensor("xT_k", [F, S], F32, kind="ExternalInput")
    t_xbf = nc.dram_tensor("xbf", [N, F], BF16, kind="ExternalInput")
    t_eaT = nc.dram_tensor("eaT_k", [F, S], F32, kind="ExternalInput")
    t_lct = nc.dram_tensor("lct_k", [N, S], BF16, kind="ExternalInput")
    t_lcn = nc.dram_tensor("lcn_k", [E, S], BF16, kind="ExternalInput")
    t_wt = [nc.dram_tensor(f"w{i}t", [F, F], F32, kind="ExternalInput") for i in (1, 2)]
    t_fct = [nc.dram_tensor(f"fc{i}t", [F, HID], F32, kind="ExternalInput") for i in (1, 2)]
    t_a1wt = nc.dram_tensor("a1wt_k", [N, S], BF16, kind="ExternalInput")
    t_attx = [nc.dram_tensor(f"attx{i}", [1, F], F32, kind="ExternalInput") for i in (1, 2)]
    t_atte = [nc.dram_tensor(f"atte{i}", [1, F], F32, kind="ExternalInput") for i in (1, 2)]
    t_dvec = nc.dram_tensor("dvec_k", [1, S], F32, kind="ExternalInput")
    t_bvk = nc.dram_tensor("bvec_k", [128, NT], F32, kind="ExternalInput")
    t_hgb = [nc.dram_tensor(f"hgb{i}", [128, KT], F32, kind="ExternalInput") for i in (1, 2)]
    t_gn = [nc.dram_tensor(f"gn{i}", [128, 3 * KT], F32, kind="ExternalInput") for i in (1, 2)]
    t_fcb = [nc.dram_tensor(f"fcb{i}", [128, NT], F32, kind="ExternalInput") for i in (1, 2)]
    t_fcbr = [nc.dram_tensor(f"fcb{i}r", [1, HID], F32, kind="ExternalInput") for i in (1, 2)]
    t_a1b = nc.dram_tensor("a1b_k", [128, NT], F32, kind="ExternalInput")
    t_a2w = nc.dram_tensor("a2w_k", [128, NT], F32, kind="ExternalInput")
    t_a2b = nc.dram_tensor("a2b", [1, 1], F32, kind="ExternalInput")
    t_clsw = nc.dram_tensor("clsw", [2 * F, 4], F32, kind="ExternalInput")
    t_clsb = nc.dram_tensor("clsb", [1, 4], F32, kind="ExternalInput")

    t_y = nc.dram_tensor("y", [S, 4], F32, kind="ExternalOutput")

    # ------------- internal DRAM + collective buffers -------------
    b_xw = [nc.dram_tensor(f"xw{i}_b", [S, F], F32) for i in (1, 2)]
    g_xw = [nc.dram_tensor(f"xw{i}_g", [N, F], F32, addr_space="Shared") for i in (1, 2)]
    b_ax = [nc.dram_tensor(f"ax{i}_b", [S, 1], F32) for i in (1, 2)]
    g_ax = [nc.dram_tensor(f"ax{i}_g", [N, 1], F32, addr_space="Shared") for i in (1, 2)]
    b_ae = [nc.dram_tensor(f"ae{i}_b", [S, 1], F32) for i in (1, 2)]
    g_ae = [nc.dram_tensor(f"ae{i}_g", [N, 1], F32, addr_space="Shared") for i in (1, 2)]
    b_m = [nc.dram_tensor(f"m{i}_b", [S, F], F32) for i in (1, 2)]
    g_m = [nc.dram_tensor(f"m{i}_g", [N, F], F32, addr_space="Shared") for i in (1, 2)]
    b_gns = [nc.dram_tensor(f"gns{i}_b", [128, 2 * KT], F32) for i in (1, 2)]
    g_gns = [nc.dram_tensor(f"gns{i}_g", [128, 2 * KT], F32, addr_space="Shared") for i in (1, 2)]
    b_o = [nc.dram_tensor(f"o{i}_b", [S, HID], BF16) for i in (1, 2)]
    g_o = [nc.dram_tensor(f"o{i}_g", [N, HID], BF16, addr_space="Shared") for i in (1, 2)]
    b_s = nc.dram_tensor("s_b", [128, 16], F32)
    g_s = nc.dram_tensor("s_g", [128, 16], F32, addr_space="Shared")
    b_sm = nc.dram_tensor("sm_b", [1, 1], F32)

    RG = [list(range(NCORES))]

    def ag(bounce, out_shared):
        nc.gpsimd.collective_compute("AllGather", ALU.bypass, replica_groups=RG,
                                     ins=[bounce.ap()], outs=[out_shared.ap()])

    def ar(bounce, out_shared):
        nc.gpsimd.collective_compute("AllReduce", ALU.add, replica_groups=RG,
                                     ins=[bounce.ap()], outs=[out_shared.ap()])

    with tile.TileContext(nc) as tc:
        ctxs = []

        def pool(name, bufs, space="SBUF"):
            c = tc.tile_pool(name=name, bufs=bufs, space=space)
            p = c.__enter__()
            ctxs.append(c)
            return p

        cst = pool("cst", 1)   # persistent constants / small per-conv params
        big = pool("big", 1)   # persistent big activations
        wk = pool("wk", 3)     # streaming row tiles (generic size-keyed tags)
        sm = pool("sm", 2)     # small scratch

        ones = cst.tile([128, 1], F32)
        nc.vector.memset(ones, 1.0)
        epsc = cst.tile([128, 1], F32)
        nc.vector.memset(epsc, 1e-5)

        xT_sb = big.tile([128, KT, S], F32)
        nc.sync.dma_start(out=xT_sb, in_=t_xT.ap().rearrange("(kt p) n -> p kt n", p=128))
        eaT_sb = big.tile([128, KT, S], F32)
        nc.sync.dma_start(out=eaT_sb, in_=t_eaT.ap().rearrange("(kt p) n -> p kt n", p=128))
        h1T_sb = big.tile([128, KT, S], F32)
        o1T_sb = big.tile([128, NT, S], F32)
        o2T_sb = big.tile([128, NT, S], F32)
        oT_sb = [o1T_sb, o2T_sb]

        dbc = cst.tile([128, S], F32)
        nc.gpsimd.dma_start(out=dbc, in_=_bcast(t_dvec, 0, 1, S))
        bvk_sb = cst.tile([128, NT], F32)
        nc.sync.dma_start(out=bvk_sb, in_=t_bvk[:])

        # =========================================================
        def conv(ci, srcT_sb):
            """conv ci (0/1), input srcT_sb [128, KT, S] T-layout.
            Writes GN'd+leaky output into h1T_sb; fc outputs into oT_sb[ci]."""
            axb = cst.tile([128, F], F32, tag="axb", name=f"axb{ci}")
            nc.gpsimd.dma_start(out=axb, in_=_bcast(t_attx[ci], 0, 1, F))
            aeb = cst.tile([128, F], F32, tag="aeb", name=f"aeb{ci}")
            nc.gpsimd.dma_start(out=aeb, in_=_bcast(t_atte[ci], 0, 1, F))

            # --- xw (node-major shard rows) + ax partials; kt-outer ---
            ax_sb4 = sm.tile([128, NT], F32, tag="ax4", name=f"ax4{ci}")
            ae_sb4 = sm.tile([128, NT], F32, tag="ae4", name=f"ae4{ci}")
            with tc.tile_pool(name=f"psAx{ci}", bufs=1, space="PSUM") as pA:
                pxw = [pA.tile([128, 512], F32, tag=f"pxw{i}", name=f"pxw{ci}_{i}")
                       for i in range(8)]
                for kt in range(KT):
                    wtr = wk.tile([128, F], F32, tag="row_f", name=f"wa{ci}_{kt}")
                    nc.sync.dma_start(out=wtr, in_=t_wt[ci][kt * 128:(kt + 1) * 128, :])
                    for nt in range(NT):
                        for fo in range(2):
                            nc.tensor.matmul(pxw[nt * 2 + fo],
                                             srcT_sb[:, kt, nt * 128:(nt + 1) * 128],
                                             wtr[:, fo * 512:(fo + 1) * 512],
                                             start=(kt == 0), stop=(kt == KT - 1))
                for nt in range(NT):
                    xwr = wk.tile([128, F], F32, tag="row_f2", name=f"xwr{ci}_{nt}")
                    nc.vector.tensor_copy(xwr[:, 0:512], pxw[nt * 2])
                    nc.vector.tensor_copy(xwr[:, 512:F], pxw[nt * 2 + 1])
                    nc.sync.dma_start(out=b_xw[ci][nt * 128:(nt + 1) * 128, :], in_=xwr)
                    tmp = wk.tile([128, F], F32, tag="row_f3", name=f"axt{ci}_{nt}")
                    nc.vector.tensor_tensor(tmp, xwr, axb, op=ALU.mult)
                    nc.vector.reduce_sum(ax_sb4[:, nt:nt + 1], tmp, axis=AX)
            # --- ew -> ae partials; kt-outer ---
            with tc.tile_pool(name=f"psAe{ci}", bufs=1, space="PSUM") as pA:
                pew = [pA.tile([128, 512], F32, tag=f"pew{i}", name=f"pew{ci}_{i}")
                       for i in range(8)]
                for kt in range(KT):
                    wtr = wk.tile([128, F], F32, tag="row_f", name=f"we{ci}_{kt}")
                    nc.sync.dma_start(out=wtr, in_=t_wt[ci][kt * 128:(kt + 1) * 128, :])
                    for et in range(NT):
                        for fo in range(2):
                            nc.tensor.matmul(pew[et * 2 + fo],
                                             eaT_sb[:, kt, et * 128:(et + 1) * 128],
                                             wtr[:, fo * 512:(fo + 1) * 512],
                                             start=(kt == 0), stop=(kt == KT - 1))
                for et in range(NT):
                    first = True
                    for fo in range(2):
                        tmp = wk.tile([128, 512], F32, tag="row_s", name=f"aet{ci}_{et}_{fo}")
                        nc.vector.tensor_tensor(tmp, pew[et * 2 + fo],
                                                aeb[:, fo * 512:(fo + 1) * 512], op=ALU.mult)
                        rr = sm.tile([128, 1], F32, tag="aer", name=f"aer{ci}_{et}_{fo}")
                        nc.vector.reduce_sum(rr, tmp, axis=AX)
                        if first:
                            nc.vector.tensor_copy(ae_sb4[:, et:et + 1], rr)
                            first = False
                        else:
                            nc.vector.tensor_tensor(ae_sb4[:, et:et + 1],
                                                    ae_sb4[:, et:et + 1], rr, op=ALU.add)
            nc.sync.dma_start(out=b_ax[ci].ap().rearrange("(nt p) 1 -> p nt", p=128),
                              in_=ax_sb4)
            nc.sync.dma_start(out=b_ae[ci].ap().rearrange("(nt p) 1 -> p nt", p=128),
                              in_=ae_sb4)
            ag(b_xw[ci], g_xw[ci])
            ag(b_ax[ci], g_ax[ci])
            ag(b_ae[ci], g_ae[ci])

            # --- m-phase ---
            ax_pk = cst.tile([128, NK], F32, tag="ax_pk", name=f"ax_pk{ci}")
            nc.sync.dma_start(out=ax_pk,
                              in_=g_ax[ci].ap().rearrange("(i p) 1 -> p i", p=128))
            aeb_loc = cst.tile([128, S], F32, tag="aeb_loc", name=f"aeb_loc{ci}")
            nc.gpsimd.dma_start(out=aeb_loc, in_=_bcast(b_ae[ci], 0, 1, S))
            acc = big.tile([128, S], F32, tag="acc", name=f"acc{ci}")
            nc.vector.memset(acc, 0.0)
            m_sb = big.tile([128, NT, F], F32, tag="mh", name=f"m_sb{ci}")
            with tc.tile_pool(name=f"psM{ci}", bufs=1, space="PSUM") as pM:
                mps = [pM.tile([128, 512], F32, tag=f"mps{i}", name=f"mps{ci}_{i}")
                       for i in range(8)]
                for nk in range(NK):
                    xwt = wk.tile([128, F], F32, tag="row_f", name=f"mxw{ci}_{nk}")
                    nc.sync.dma_start(out=xwt, in_=g_xw[ci][nk * 128:(nk + 1) * 128, :])
                    lctt = wk.tile([128, S], BF16, tag="row_sb", name=f"mlc{ci}_{nk}")
                    nc.sync.dma_start(out=lctt, in_=t_lct[nk * 128:(nk + 1) * 128, :])
                    z = wk.tile([128, S], F32, tag="row_s", name=f"mz{ci}_{nk}")
                    nc.scalar.activation(z, aeb_loc, AF.Prelu,
                                         bias=ax_pk[:, nk:nk + 1], alpha=0.2)
                    nc.vector.tensor_tensor(z, z, lctt, op=ALU.add)
                    nc.scalar.activation(z, z, AF.Exp)
                    nc.vector.tensor_tensor(acc, acc, z, op=ALU.add)
                    for et in range(NT):
                        for fo in range(2):
                            nc.tensor.matmul(mps[et * 2 + fo],
                                             z[:, et * 128:(et + 1) * 128],
                                             xwt[:, fo * 512:(fo + 1) * 512],
                                             start=(nk == 0), stop=(nk == NK - 1))
                for et in range(NT):
                    for fo in range(2):
                        nc.vector.tensor_copy(m_sb[:, et, fo * 512:(fo + 1) * 512],
                                              mps[et * 2 + fo])
            with tc.tile_pool(name=f"psD{ci}", bufs=1, space="PSUM") as pD:
                dps = pD.tile([128, NT], F32, name=f"dps{ci}")
                for et in range(NT):
                    nc.tensor.matmul(dps[:, et:et + 1], acc[:, et * 128:(et + 1) * 128],
                                     ones, start=True, stop=True)
                den = sm.tile([128, NT], F32, tag="den", name=f"den{ci}")
                nc.vector.tensor_scalar(den, dps, 1e-16, None, op0=ALU.add)
            rec = sm.tile([128, NT], F32, tag="rec", name=f"rec{ci}")
            nc.vector.reciprocal(rec, den)
            sc = sm.tile([128, NT], F32, tag="sc", name=f"sc{ci}")
            nc.vector.tensor_tensor(sc, rec, rec, op=ALU.mult)
            nc.vector.tensor_tensor(sc, sc, bvk_sb, op=ALU.mult)
            for et in range(NT):
                nc.vector.tensor_scalar(m_sb[:, et, :], m_sb[:, et, :],
                                        sc[:, et:et + 1], None, op0=ALU.mult)
                nc.sync.dma_start(out=b_m[ci][et * 128:(et + 1) * 128, :],
                                  in_=m_sb[:, et, :])
            ag(b_m[ci], g_m[ci])

            # --- out-phase + GraphNorm + leaky ---
            ae_pk = cst.tile([128, NK], F32, tag="ae_pk", name=f"ae_pk{ci}")
            nc.sync.dma_start(out=ae_pk,
                              in_=g_ae[ci].ap().rearrange("(i p) 1 -> p i", p=128))
            axb_loc = cst.tile([128, S], F32, tag="axb_loc", name=f"axb_loc{ci}")
            nc.gpsimd.dma_start(out=axb_loc, in_=_bcast(b_ax[ci], 0, 1, S))
            hgb_sb = cst.tile([128, KT], F32, tag="hgb", name=f"hgb_sb{ci}")
            nc.sync.dma_start(out=hgb_sb, in_=t_hgb[ci][:])
            hpre = big.tile([128, KT, S], F32, tag="hpre", name=f"hpre{ci}")
            s12 = sm.tile([128, 2 * KT], F32, tag="s12", name=f"s12{ci}")
            with tc.tile_pool(name=f"psO{ci}", bufs=1, space="PSUM") as pO:
                ops_ = [pO.tile([128, 512], F32, tag=f"ops{i}", name=f"ops{ci}_{i}")
                        for i in range(KT)]
                for ek in range(NK):
                    mlh = wk.tile([128, F], F32, tag="row_f", name=f"om{ci}_{ek}")
                    nc.sync.dma_start(out=mlh, in_=g_m[ci][ek * 128:(ek + 1) * 128, :])
                    lcnt = wk.tile([128, S], BF16, tag="row_sb", name=f"olc{ci}_{ek}")
                    nc.sync.dma_start(out=lcnt, in_=t_lcn[ek * 128:(ek + 1) * 128, :])
                    zo = wk.tile([128, S], F32, tag="row_s", name=f"oz{ci}_{ek}")
                    nc.scalar.activation(zo, axb_loc, AF.Prelu,
                                         bias=ae_pk[:, ek:ek + 1], alpha=0.2)
                    nc.vector.tensor_tensor(zo, zo, lcnt, op=ALU.add)
                    nc.scalar.activation(zo, zo, AF.Exp)
                    for ft in range(KT):
                        nc.tensor.matmul(ops_[ft], mlh[:, ft * 128:(ft + 1) * 128], zo,
                                         start=(ek == 0), stop=(ek == NK - 1))
                for ft in range(KT):
                    nc.vector.tensor_tensor(hpre[:, ft, :], ops_[ft], dbc, op=ALU.mult)
                    nc.vector.tensor_scalar(hpre[:, ft, :], hpre[:, ft, :],
                                            hgb_sb[:, ft:ft + 1], None, op0=ALU.add)
                    nc.vector.reduce_sum(s12[:, ft:ft + 1], hpre[:, ft, :], axis=AX)
                    sq = wk.tile([128, S], F32, tag="row_s", name=f"sq{ci}_{ft}")
                    nc.vector.tensor_tensor(sq, hpre[:, ft, :], hpre[:, ft, :], op=ALU.mult)
                    nc.vector.reduce_sum(s12[:, KT + ft:KT + ft + 1], sq, axis=AX)
            nc.sync.dma_start(out=b_gns[ci][:], in_=s12)
            ar(b_gns[ci], g_gns[ci])
            gs = sm.tile([128, 2 * KT], F32, tag="gs", name=f"gs{ci}")
            nc.sync.dma_start(out=gs, in_=g_gns[ci][:])
            gnp = cst.tile([128, 3 * KT], F32, tag="gnp", name=f"gnp{ci}")
            nc.sync.dma_start(out=gnp, in_=t_gn[ci][:])
            mean = sm.tile([128, KT], F32, tag="mean", name=f"mean{ci}")
            nc.vector.tensor_scalar(mean, gs[:, 0:KT], 1.0 / N, None, op0=ALU.mult)
            means = sm.tile([128, KT], F32, tag="means", name=f"means{ci}")
            nc.vector.tensor_tensor(means, mean, gnp[:, 2 * KT:3 * KT], op=ALU.mult)
            var = sm.tile([128, KT], F32, tag="var", name=f"var{ci}")
            nc.vector.tensor_scalar(var, gs[:, KT:2 * KT], 1.0 / N, None, op0=ALU.mult)
            tmpv = sm.tile([128, KT], F32, tag="tmpv", name=f"tmpv{ci}")
            nc.vector.tensor_tensor(tmpv, means, mean, op=ALU.mult)
            nc.vector.tensor_scalar(tmpv, tmpv, 2.0, None, op0=ALU.mult)
            nc.vector.tensor_tensor(var, var, tmpv, op=ALU.subtract)
            nc.vector.tensor_tensor(tmpv, means, means, op=ALU.mult)
            nc.vector.tensor_tensor(var, var, tmpv, op=ALU.add)
            rstd = sm.tile([128, KT], F32, tag="rstd", name=f"rstd{ci}")
            nc.scalar.activation(rstd, var, AF.Sqrt, bias=epsc)
            nc.vector.reciprocal(rstd, rstd)
            gsc = sm.tile([128, KT], F32, tag="gsc", name=f"gsc{ci}")
            nc.vector.tensor_tensor(gsc, gnp[:, 0:KT], rstd, op=ALU.mult)
            gsh = sm.tile([128, KT], F32, tag="gsh", name=f"gsh{ci}")
            nc.vector.tensor_tensor(gsh, means, gsc, op=ALU.mult)
            nc.vector.tensor_tensor(gsh, gnp[:, KT:2 * KT], gsh, op=ALU.subtract)
            for ft in range(KT):
                nc.scalar.activation(h1T_sb[:, ft, :], hpre[:, ft, :], AF.Lrelu,
                                     bias=gsh[:, ft:ft + 1], scale=gsc[:, ft:ft + 1])

            # --- fc (both orientations), kt-outer single pass over fct ---
            fcb_sb = cst.tile([128, NT], F32, tag="fcb", name=f"fcb_sb{ci}")
            nc.sync.dma_start(out=fcb_sb, in_=t_fcb[ci][:])
            fcbb = cst.tile([128, HID], F32, tag="fcbb", name=f"fcbb{ci}")
            nc.gpsimd.dma_start(out=fcbb, in_=_bcast(t_fcbr[ci], 0, 1, HID))
            with tc.tile_pool(name=f"psF{ci}", bufs=1, space="PSUM") as pF:
                pf1 = [pF.tile([128, S], F32, tag=f"pf1_{i}", name=f"pf1{ci}_{i}")
                       for i in range(NT)]
                pf2 = [pF.tile([128, HID], F32, tag=f"pf2_{i}", name=f"pf2{ci}_{i}")
                       for i in range(NT)]
                for kt in range(KT):
                    fcr = wk.tile([128, HID], F32, tag="row_h", name=f"fcr{ci}_{kt}")
                    nc.sync.dma_start(out=fcr, in_=t_fct[ci][kt * 128:(kt + 1) * 128, :])
                    for hot in range(NT):
                        nc.tensor.matmul(pf1[hot], fcr[:, hot * 128:(hot + 1) * 128],
                                         h1T_sb[:, kt, :],
                                         start=(kt == 0), stop=(kt == KT - 1))
                    for nt in range(NT):
                        nc.tensor.matmul(pf2[nt], h1T_sb[:, kt, nt * 128:(nt + 1) * 128],
                                         fcr, start=(kt == 0), stop=(kt == KT - 1))
                for hot in range(NT):
                    nc.scalar.activation(oT_sb[ci][:, hot, :], pf1[hot], AF.Lrelu,
                                         bias=fcb_sb[:, hot:hot + 1])
                for nt in range(NT):
                    tmpo = wk.tile([128, HID], F32, tag="row_h", name=f"ot{ci}_{nt}")
                    nc.vector.tensor_tensor(tmpo, pf2[nt], fcbb, op=ALU.add)
                    onm = wk.tile([128, HID], BF16, tag="row_hb", name=f"onm{ci}_{nt}")
                    nc.scalar.activation(onm, tmpo, AF.Lrelu)
                    nc.sync.dma_start(out=b_o[ci][nt * 128:(nt + 1) * 128, :], in_=onm)
            ag(b_o[ci], g_o[ci])

        conv(0, xT_sb)
        conv(1, h1T_sb)

        # ===================== attention head =====================
        a1b_sb = cst.tile([128, NT], F32)
        nc.sync.dma_start(out=a1b_sb, in_=t_a1b[:])
        a2w_sb = cst.tile([128, NT], F32)
        nc.sync.dma_start(out=a2w_sb, in_=t_a2w[:])
        s_acc = big.tile([128, 4, 512], F32)
        for rnd in range(2):
            with tc.tile_pool(name=f"psQ{rnd}", bufs=1, space="PSUM") as pQ:
                qps = [pQ.tile([128, 512], F32, tag=f"qps{i}", name=f"qps{rnd}_{i}")
                       for i in range(8)]
                for nk in range(NK):
                    rhs = wk.tile([128, F], BF16, tag="row_fb", name=f"qr{rnd}_{nk}")
                    if rnd == 0:
                        nc.sync.dma_start(out=rhs, in_=t_xbf[nk * 128:(nk + 1) * 128, :])
                    else:
                        nc.sync.dma_start(out=rhs[:, 0:HID],
                                          in_=g_o[0][nk * 128:(nk + 1) * 128, :])
                        nc.sync.dma_start(out=rhs[:, HID:F],
                                          in_=g_o[1][nk * 128:(nk + 1) * 128, :])
                    lhs = wk.tile([128, S], BF16, tag="row_sb", name=f"ql{rnd}_{nk}")
                    nc.sync.dma_start(out=lhs, in_=t_a1wt[nk * 128:(nk + 1) * 128, :])
                    for jt in range(NT):
                        for cb in range(2):
                            nc.tensor.matmul(qps[jt * 2 + cb],
                                             lhs[:, jt * 128:(jt + 1) * 128],
                                             rhs[:, cb * 512:(cb + 1) * 512],
                                             start=(nk == 0), stop=(nk == NK - 1))
                for jt in range(NT):
                    for cb in range(2):
                        zq = wk.tile([128, 512], F32, tag="row_s", name=f"zq{rnd}_{jt}_{cb}")
                        nc.scalar.activation(zq, qps[jt * 2 + cb], AF.Relu,
                                             bias=a1b_sb[:, jt:jt + 1])
                        nc.vector.tensor_scalar(zq, zq, a2w_sb[:, jt:jt + 1],
                                                None, op0=ALU.mult)
                        if jt == 0:
                            nc.vector.tensor_copy(s_acc[:, rnd * 2 + cb, :], zq)
                        else:
                            nc.vector.tensor_tensor(s_acc[:, rnd * 2 + cb, :],
                                                    s_acc[:, rnd * 2 + cb, :], zq,
                                                    op=ALU.add)
        s_sb = sm.tile([128, 16], F32)
        with tc.tile_pool(name="psS", bufs=1, space="PSUM") as pS:
            sps = pS.tile([128, 16], F32)
            sflat = s_acc.rearrange("p a c -> p (a c)")
            for ct in range(16):
                nc.tensor.matmul(sps[:, ct:ct + 1], sflat[:, ct * 128:(ct + 1) * 128],
                                 ones, start=True, stop=True)
            nc.vector.tensor_copy(s_sb, sps)
        nc.sync.dma_start(out=b_s[:], in_=s_sb)
        ar(b_s, g_s)
        ss = sm.tile([128, 16], F32)
        nc.sync.dma_start(out=ss, in_=g_s[:])
        a2bb = cst.tile([128, 1], F32)
        nc.gpsimd.dma_start(out=a2bb, in_=_bcast(t_a2b, 0, 1, 1))
        nc.vector.tensor_scalar(ss, ss, a2bb, None, op0=ALU.add)
        nc.scalar.activation(ss, ss, AF.Sigmoid)
        srow = sm.tile([128, 1], F32)
        nc.vector.reduce_sum(srow, ss, axis=AX)
        with tc.tile_pool(name="psSM", bufs=1, space="PSUM") as pSM:
            smps = pSM.tile([1, 1], F32)
            nc.tensor.matmul(smps, srow, ones, start=True, stop=True)
            smt = sm.tile([1, 1], F32)
            nc.vector.tensor_copy(smt, smps)
        nc.sync.dma_start(out=b_sm[:], in_=smt)
        smb = sm.tile([128, 1], F32)
        nc.gpsimd.dma_start(out=smb, in_=_bcast(b_sm, 0, 1, 1))
        nc.vector.tensor_scalar(smb, smb, 1.0 / (2 * F), None, op0=ALU.mult)
        nc.vector.tensor_scalar(ss, ss, smb, None, op0=ALU.subtract)

        clsw_sb = cst.tile([128, 16, 4], F32)
        nc.sync.dma_start(out=clsw_sb, in_=t_clsw.ap().rearrange("(ct p) o -> p ct o", p=128))
        for ct in range(16):
            nc.vector.tensor_scalar(clsw_sb[:, ct, :], clsw_sb[:, ct, :],
                                    ss[:, ct:ct + 1], None, op0=ALU.mult)
        clsbb = cst.tile([128, 4], F32)
        nc.gpsimd.dma_start(out=clsbb, in_=_bcast(t_clsb, 0, 1, 4))
        lg_sb = sm.tile([128, NT, 4], F32)
        with tc.tile_pool(name="psL", bufs=2, space="PSUM") as pL:
            for nt in range(NT):
                ps = pL.tile([128, 4], F32, tag="psl", name=f"psl{nt}")
                for ct in range(16):
                    if ct < 8:
                        lhsT = xT_sb[:, ct, nt * 128:(nt + 1) * 128]
                    elif ct < 12:
                        lhsT = o1T_sb[:, ct - 8, nt * 128:(nt + 1) * 128]
                    else:
                        lhsT = o2T_sb[:, ct - 12, nt * 128:(nt + 1) * 128]
                    nc.tensor.matmul(ps, lhsT, clsw_sb[:, ct, :],
                                     start=(ct == 0), stop=(ct == 15))
                nc.vector.tensor_tensor(lg_sb[:, nt, :], ps, clsbb, op=ALU.add)
        nc.sync.dma_start(out=t_y.ap().rearrange("(nt p) o -> p nt o", p=128), in_=lg_sb)

        for c in reversed(ctxs):
            c.__exit__(None, None, None)

    nc.compile()
    return nc


# ====================== host side ======================

def _preprocess(inputs):
    x = np.ascontiguousarray(np.asarray(inputs["x"], np.float32))
    ea = np.ascontiguousarray(np.asarray(inputs["edge_attr"], np.float32))
    ei = np.asarray(inputs["edge_index"])
    row = np.asarray(ei[0], np.int64)
    col = np.asarray(ei[1], np.int64)

    C = np.zeros((E, N), np.float32)
    np.add.at(C, (col, row), 1.0)
    LC = np.where(C > 0, np.log(np.maximum(C, 1e-30)), -60.0).astype(np.float32)
    deg_n = np.bincount(row, minlength=N).astype(np.float32)
    deg_e = np.bincount(col, minlength=E).astype(np.float32)
    D = np.where(deg_n > 0, 1.0 / np.maximum(deg_n, 1), 0.0).astype(np.float32)
    B = np.where(deg_e > 0, 1.0 / np.maximum(deg_e, 1), 0.0).astype(np.float32)

    bf = ml_dtypes.bfloat16
    LCT_bf = LC.T.astype(bf)           # [N, E]
    LC_bf = LC.astype(bf)              # [E, N]
    xbf = x.astype(bf)
    a1w = np.asarray(inputs["att1_W"], np.float32)

    def pack_pp(v, nt):  # [nt*128] -> [128, nt]
        return np.ascontiguousarray(v.reshape(nt, 128).T.astype(np.float32))

    f32 = np.float32
    com = {
        "xbf": xbf,
        "w1t": np.ascontiguousarray(np.asarray(inputs["hg1_W"], f32).T),
        "w2t": np.ascontiguousarray(np.asarray(inputs["hg2_W"], f32).T),
        "fc1t": np.ascontiguousarray(np.asarray(inputs["fc1_W"], f32).T),
        "fc2t": np.ascontiguousarray(np.asarray(inputs["fc2_W"], f32).T),
        "attx1": np.asarray(inputs["hg1_att"], f32)[:F].reshape(1, F),
        "atte1": np.asarray(inputs["hg1_att"], f32)[F:].reshape(1, F),
        "attx2": np.asarray(inputs["hg2_att"], f32)[:F].reshape(1, F),
        "atte2": np.asarray(inputs["hg2_att"], f32)[F:].reshape(1, F),
        "hgb1": pack_pp(np.asarray(inputs["hg1_b"], f32), KT),
        "hgb2": pack_pp(np.asarray(inputs["hg2_b"], f32), KT),
        "gn1": np.concatenate([pack_pp(np.asarray(inputs[k], f32), KT)
                               for k in ("gn1_w", "gn1_b", "gn1_ms")], axis=1),
        "gn2": np.concatenate([pack_pp(np.asarray(inputs[k], f32), KT)
                               for k in ("gn2_w", "gn2_b", "gn2_ms")], axis=1),
        "fcb1": pack_pp(np.asarray(inputs["fc1_b"], f32), NT),
        "fcb2": pack_pp(np.asarray(inputs["fc2_b"], f32), NT),
        "fcb1r": np.asarray(inputs["fc1_b"], f32).reshape(1, HID),
        "fcb2r": np.asarray(inputs["fc2_b"], f32).reshape(1, HID),
        "a2b": np.asarray(inputs["att2_b"], f32).reshape(1, 1),
        "clsw": np.ascontiguousarray(np.asarray(inputs["cls_W"], f32).T),
        "clsb": np.asarray(inputs["cls_b"], f32).reshape(1, 4),
    }
    att1_b = np.asarray(inputs["att1_b"], f32)
    att2_w = np.asarray(inputs["att2_W"], f32)[0]

    in_maps = []
    for k in range(NCORES):
        sl = slice(k * S, (k + 1) * S)
        m = dict(com)
        m["xT_k"] = np.ascontiguousarray(x[sl].T)
        m["eaT_k"] = np.ascontiguousarray(ea[sl].T)
        m["lct_k"] = np.ascontiguousarray(LCT_bf[:, sl])
        m["lcn_k"] = np.ascontiguousarray(LC_bf[:, sl])
        m["a1wt_k"] = np.ascontiguousarray(a1w[sl].T.astype(bf))
        m["dvec_k"] = D[sl].reshape(1, S).copy()
        m["bvec_k"] = pack_pp(B[sl], NT)
        m["a1b_k"] = pack_pp(att1_b[sl], NT)
        m["a2w_k"] = pack_pp(att2_w[sl], NT)
        in_maps.append(m)
    return in_maps


def kernel(**inputs) -> np.ndarray:
    if "nc" not in _CACHE:
        _CACHE["nc"] = build_program()
    nc = _CACHE["nc"]
    in_maps = _preprocess(inputs)
    last_err = None
    for _ in range(3):
        try:
            res = run_bass_kernel_spmd(nc, in_maps, list(range(NCORES))).results
            return np.concatenate([res[k]["y"] for k in range(NCORES)], axis=0)
        except Exception as e:  # flaky NRT_EXEC_UNIT_UNRECOVERABLE retries
            last_err = e
    raise last_err


# revision 6
# speedup vs baseline: 7550.5303x; 7550.5303x over previous
"""Trainium2 Bass kernel for nn_GCN_19791209300130 (hypergraph GCN, 8 cores).

Strategy: densify the sparse hypergraph incidence structure into a count
matrix C [E, N] built on host from edge_index (index-only preprocessing).
With LC = ln(C) (-60 for zeros), the attention-weighted scatter phases become
dense matmuls:
    CEX[e,n]   = exp(lrelu_0.2(ax[n] + ae[e]) + LC[e,n])   (= C * exp(leaky(a)))
    denom[e]   = sum_n CEX[e,n] + 1e-16
    m'[e,:]    = (B[e]/denom[e]^2) * (CEX @ xw)[e,:]
    convT[f,n] = D[n] * sum_e m'[e,f]*CEX[e,n] + bias[f]
This matches PyG HypergraphConv exactly (softmax is shift-invariant so the
segment-max shift can be dropped; logits are ~N(0,0.6), no overflow risk).

Sharding (8 cores): core k owns node slice Nk and edge slice Ek (512 each).
Per conv: xw sharded by n + AllGather; m-phase sharded by e (contract all n);
m' AllGather; out-phase sharded by n (contract all e). Attention head sharded
by att1-row j with AllReduce of the s partials; GraphNorm via AllReduced raw
moments. Matmul operands are bf16 (fp32 PE rate is ~7x slower on trn2); PSUM
accumulation and all normalization math stay fp32. Phase order interleaves
independent compute (attention round 1, fc1) under the AllGather latencies.
"""
import numpy as np
import ml_dtypes

import concourse.bass as bass
import concourse.bacc as bacc
import concourse.tile as tile
from concourse import mybir
from concourse.bass_utils import run_bass_kernel_spmd

NCORES = 8
N = 4096
E = 4096
F = 1024
HID = 512
S = N // NCORES      # 512 shard
NT = S // 128        # 4
KT = F // 128        # 8
NK = N // 128        # 32

F32 = mybir.dt.float32
BF16 = mybir.dt.bfloat16
AF = mybir.ActivationFunctionType
ALU = mybir.AluOpType
AX = mybir.AxisListType.X

_CACHE = {}


def _bcast(t, offset, step, count, parts=128):
    """DRAM AP broadcast across partitions: count elems at offset with step."""
    return bass.AP(tensor=t.ap().tensor, offset=offset,
                   ap=[[0, parts], [step, count]])


def build_program():
    nc = bacc.Bacc("TRN2", target_bir_lowering=False, debug=False,
                   num_devices=NCORES)

    # ---------------- inputs ----------------
    t_xT = nc.dram_tensor("xT_k", [F, S], BF16, kind="ExternalInput")
    t_xbf = nc.dram_tensor("xbf", [N, F], BF16, kind="ExternalInput")
    t_eaT = nc.dram_tensor("eaT_k", [F, S], BF16, kind="ExternalInput")
    t_lct = nc.dram_tensor("lct_k", [N, S], BF16, kind="ExternalInput")
    t_lcn = nc.dram_tensor("lcn_k", [E, S], BF16, kind="ExternalInput")
    t_wt = [nc.dram_tensor(f"w{i}t", [F, F], BF16, kind="ExternalInput") for i in (1, 2)]
    t_fct = [nc.dram_tensor(f"fc{i}t", [F, HID], BF16, kind="ExternalInput") for i in (1, 2)]
    t_a1wt = nc.dram_tensor("a1wt_k", [N, S], BF16, kind="ExternalInput")
    t_attx = [nc.dram_tensor(f"attx{i}", [1, F], F32, kind="ExternalInput") for i in (1, 2)]
    t_atte = [nc.dram_tensor(f"atte{i}", [1, F], F32, kind="ExternalInput") for i in (1, 2)]
    t_dvec = nc.dram_tensor("dvec_k", [1, S], F32, kind="ExternalInput")
    t_bvk = nc.dram_tensor("bvec_k", [128, NT], F32, kind="ExternalInput")
    t_hgb = [nc.dram_tensor(f"hgb{i}", [128, KT], F32, kind="ExternalInput") for i in (1, 2)]
    t_gn = [nc.dram_tensor(f"gn{i}", [128, 3 * KT], F32, kind="ExternalInput") for i in (1, 2)]
    t_fcb = [nc.dram_tensor(f"fcb{i}", [128, NT], F32, kind="ExternalInput") for i in (1, 2)]
    t_fcbr = [nc.dram_tensor(f"fcb{i}r", [1, HID], F32, kind="ExternalInput") for i in (1, 2)]
    t_a1b = nc.dram_tensor("a1b_k", [128, NT], F32, kind="ExternalInput")
    t_a2w = nc.dram_tensor("a2w_k", [128, NT], F32, kind="ExternalInput")
    t_a2b = nc.dram_tensor("a2b", [1, 1], F32, kind="ExternalInput")
    t_clsw = nc.dram_tensor("clsw", [2 * F, 4], F32, kind="ExternalInput")
    t_clsb = nc.dram_tensor("clsb", [1, 4], F32, kind="ExternalInput")

    t_y = nc.dram_tensor("y", [S, 4], F32, kind="ExternalOutput")

    # ------------- internal DRAM + collective buffers -------------
    b_xw = [nc.dram_tensor(f"xw{i}_b", [S, F], BF16) for i in (1, 2)]
    g_xw = [nc.dram_tensor(f"xw{i}_g", [N, F], BF16, addr_space="Shared") for i in (1, 2)]
    b_ax = [nc.dram_tensor(f"ax{i}_b", [S, 1], F32) for i in (1, 2)]
    g_ax = [nc.dram_tensor(f"ax{i}_g", [N, 1], F32, addr_space="Shared") for i in (1, 2)]
    b_ae = [nc.dram_tensor(f"ae{i}_b", [S, 1], F32) for i in (1, 2)]
    g_ae = [nc.dram_tensor(f"ae{i}_g", [N, 1], F32, addr_space="Shared") for i in (1, 2)]
    b_m = [nc.dram_tensor(f"m{i}_b", [S, F], BF16) for i in (1, 2)]
    g_m = [nc.dram_tensor(f"m{i}_g", [N, F], BF16, addr_space="Shared") for i in (1, 2)]
    b_gns = [nc.dram_tensor(f"gns{i}_b", [128, 2 * KT], F32) for i in (1, 2)]
    g_gns = [nc.dram_tensor(f"gns{i}_g", [128, 2 * KT], F32, addr_space="Shared") for i in (1, 2)]
    b_o = [nc.dram_tensor(f"o{i}_b", [S, HID], BF16) for i in (1, 2)]
    g_o = [nc.dram_tensor(f"o{i}_g", [N, HID], BF16, addr_space="Shared") for i in (1, 2)]
    b_s = nc.dram_tensor("s_b", [128, 16], F32)
    g_s = nc.dram_tensor("s_g", [128, 16], F32, addr_space="Shared")
    b_sm = nc.dram_tensor("sm_b", [1, 1], F32)

    RG = [list(range(NCORES))]

    def ag(bounce, out_shared):
        nc.gpsimd.collective_compute("AllGather", ALU.bypass, replica_groups=RG,
                                     ins=[bounce.ap()], outs=[out_shared.ap()])

    def ar(bounce, out_shared):
        nc.gpsimd.collective_compute("AllReduce", ALU.add, replica_groups=RG,
                                     ins=[bounce.ap()], outs=[out_shared.ap()])

    with tile.TileContext(nc) as tc:
        ctxs = []

        def pool(name, bufs, space="SBUF"):
            c = tc.tile_pool(name=name, bufs=bufs, space=space)
            p = c.__enter__()
            ctxs.append(c)
            return p

        cst = pool("cst", 1)   # persistent constants / per-conv params
        big = pool("big", 1)   # persistent big activations
        wk = pool("wk", 3)     # streaming row tiles
        sm = pool("sm", 2)     # small scratch

        ones = cst.tile([128, 1], F32)
        nc.vector.memset(ones, 1.0)
        epsc = cst.tile([128, 1], F32)
        nc.vector.memset(epsc, 1e-5)

        xT_sb = big.tile([128, KT, S], BF16)
        nc.sync.dma_start(out=xT_sb, in_=t_xT.ap().rearrange("(kt p) n -> p kt n", p=128))
        eaT_sb = big.tile([128, KT, S], BF16)
        nc.sync.dma_start(out=eaT_sb, in_=t_eaT.ap().rearrange("(kt p) n -> p kt n", p=128))
        h1T_sb = big.tile([128, KT, S], BF16)
        o1T_sb = big.tile([128, NT, S], BF16)
        o2T_sb = big.tile([128, NT, S], BF16)
        oT_sb = [o1T_sb, o2T_sb]
        s_acc = big.tile([128, 4, 512], F32)

        dbc = cst.tile([128, S], F32)
        nc.gpsimd.dma_start(out=dbc, in_=_bcast(t_dvec, 0, 1, S))
        bvk_sb = cst.tile([128, NT], F32)
        nc.sync.dma_start(out=bvk_sb, in_=t_bvk[:])
        a1b_sb = cst.tile([128, NT], F32)
        nc.sync.dma_start(out=a1b_sb, in_=t_a1b[:])
        a2w_sb = cst.tile([128, NT], F32)
        nc.sync.dma_start(out=a2w_sb, in_=t_a2w[:])

        def packed_load(dst32, g_src, tagp, namep):
            """Load [N,1] f32 DRAM vec into [128, NK] SBUF as dst[p,i]=v[i*128+p]
            via contiguous [32,128] load + 4 DVE 32x32 block transposes."""
            lin = sm.tile([32, 128], F32, tag=tagp, name=namep)
            nc.sync.dma_start(out=lin, in_=g_src.ap().rearrange("(q f) 1 -> q f", q=32))
            for j in range(4):
                nc.vector.transpose(dst32[32 * j:32 * (j + 1), :],
                                    lin[:, 32 * j:32 * (j + 1)])

        # =========================================================
        def conv_A(ci, srcT_sb):
            """xw (shard rows, bf16) + ax/ae; AllGathers issued at end."""
            axb = cst.tile([128, F], F32, tag="axb", name=f"axb{ci}")
            nc.gpsimd.dma_start(out=axb, in_=_bcast(t_attx[ci], 0, 1, F))
            aeb = cst.tile([128, F], F32, tag="aeb", name=f"aeb{ci}")
            nc.gpsimd.dma_start(out=aeb, in_=_bcast(t_atte[ci], 0, 1, F))

            ax_sb4 = sm.tile([128, NT], F32, tag="ax4", name=f"ax4{ci}")
            ae_sb4 = sm.tile([128, NT], F32, tag="ae4", name=f"ae4{ci}")
            with tc.tile_pool(name=f"psAx{ci}", bufs=1, space="PSUM") as pA:
                pxw = [pA.tile([128, 512], F32, tag=f"pxw{i}", name=f"pxw{ci}_{i}")
                       for i in range(8)]
                for kt in range(KT):
                    wtr = wk.tile([128, F], BF16, tag="row_fb", name=f"wa{ci}_{kt}")
                    nc.sync.dma_start(out=wtr, in_=t_wt[ci][kt * 128:(kt + 1) * 128, :])
                    for nt in range(NT):
                        for fo in range(2):
                            nc.tensor.matmul(pxw[nt * 2 + fo],
                                             srcT_sb[:, kt, nt * 128:(nt + 1) * 128],
                                             wtr[:, fo * 512:(fo + 1) * 512],
                                             start=(kt == 0), stop=(kt == KT - 1))
                for nt in range(NT):
                    xwr = wk.tile([128, F], BF16, tag="row_fb2", name=f"xwr{ci}_{nt}")
                    nc.vector.tensor_copy(xwr[:, 0:512], pxw[nt * 2])
                    nc.vector.tensor_copy(xwr[:, 512:F], pxw[nt * 2 + 1])
                    nc.sync.dma_start(out=b_xw[ci][nt * 128:(nt + 1) * 128, :], in_=xwr)
                    tmp = wk.tile([128, F], F32, tag="row_f3", name=f"axt{ci}_{nt}")
                    nc.vector.tensor_tensor(tmp[:, 0:512], pxw[nt * 2], axb[:, 0:512], op=ALU.mult)
                    nc.vector.tensor_tensor(tmp[:, 512:F], pxw[nt * 2 + 1],
                                            axb[:, 512:F], op=ALU.mult)
                    nc.vector.reduce_sum(ax_sb4[:, nt:nt + 1], tmp, axis=AX)
            with tc.tile_pool(name=f"psAe{ci}", bufs=1, space="PSUM") as pA:
                pew = [pA.tile([128, 512], F32, tag=f"pew{i}", name=f"pew{ci}_{i}")
                       for i in range(8)]
                for kt in range(KT):
                    wtr = wk.tile([128, F], BF16, tag="row_fb", name=f"we{ci}_{kt}")
                    nc.sync.dma_start(out=wtr, in_=t_wt[ci][kt * 128:(kt + 1) * 128, :])
                    for et in range(NT):
                        for fo in range(2):
                            nc.tensor.matmul(pew[et * 2 + fo],
                                             eaT_sb[:, kt, et * 128:(et + 1) * 128],
                                             wtr[:, fo * 512:(fo + 1) * 512],
                                             start=(kt == 0), stop=(kt == KT - 1))
                for et in range(NT):
                    first = True
                    for fo in range(2):
                        tmp = wk.tile([128, 512], F32, tag="row_s", name=f"aet{ci}_{et}_{fo}")
                        nc.vector.tensor_tensor(tmp, pew[et * 2 + fo],
                                                aeb[:, fo * 512:(fo + 1) * 512], op=ALU.mult)
                        rr = sm.tile([128, 1], F32, tag="aer", name=f"aer{ci}_{et}_{fo}")
                        nc.vector.reduce_sum(rr, tmp, axis=AX)
                        if first:
                            nc.vector.tensor_copy(ae_sb4[:, et:et + 1], rr)
                            first = False
                        else:
                            nc.vector.tensor_tensor(ae_sb4[:, et:et + 1],
                                                    ae_sb4[:, et:et + 1], rr, op=ALU.add)
            nc.sync.dma_start(out=b_ax[ci].ap().rearrange("(nt p) 1 -> p nt", p=128),
                              in_=ax_sb4)
            nc.sync.dma_start(out=b_ae[ci].ap().rearrange("(nt p) 1 -> p nt", p=128),
                              in_=ae_sb4)
            ag(b_xw[ci], g_xw[ci])
            ag(b_ax[ci], g_ax[ci])
            ag(b_ae[ci], g_ae[ci])

        def conv_M(ci):
            """m-phase: m'[Ek] bf16, AllGather issued at end."""
            ax_pk = cst.tile([128, NK], F32, tag="ax_pk", name=f"ax_pk{ci}")
            packed_load(ax_pk, g_ax[ci], "pl1", f"pl_ax{ci}")
            aeb_loc = cst.tile([128, S], F32, tag="aeb_loc", name=f"aeb_loc{ci}")
            nc.gpsimd.dma_start(out=aeb_loc, in_=_bcast(b_ae[ci], 0, 1, S))
            acc = big.tile([128, S], F32, tag="acc", name=f"acc{ci}")
            nc.vector.memset(acc, 0.0)
            m_sb = big.tile([128, NT, F], F32, tag="mh", name=f"m_sb{ci}")
            with tc.tile_pool(name=f"psM{ci}", bufs=1, space="PSUM") as pM:
                mps = [pM.tile([128, 512], F32, tag=f"mps{i}", name=f"mps{ci}_{i}")
                       for i in range(8)]
                for nk in range(NK):
                    xwt = wk.tile([128, F], BF16, tag="row_fb", name=f"mxw{ci}_{nk}")
                    nc.sync.dma_start(out=xwt, in_=g_xw[ci][nk * 128:(nk + 1) * 128, :])
                    lctt = wk.tile([128, S], BF16, tag="row_sb", name=f"mlc{ci}_{nk}")
                    nc.sync.dma_start(out=lctt, in_=t_lct[nk * 128:(nk + 1) * 128, :])
                    zf = wk.tile([128, S], F32, tag="row_s", name=f"mzf{ci}_{nk}")
                    nc.scalar.activation(zf, aeb_loc, AF.Prelu,
                                         bias=ax_pk[:, nk:nk + 1], alpha=0.2)
                    nc.vector.tensor_tensor(zf, zf, lctt, op=ALU.add)
                    z = wk.tile([128, S], BF16, tag="row_sb2", name=f"mz{ci}_{nk}")
                    nc.scalar.activation(z, zf, AF.Exp)
                    nc.vector.tensor_tensor(acc, acc, z, op=ALU.add)
                    for et in range(NT):
                        for fo in range(2):
                            nc.tensor.matmul(mps[et * 2 + fo],
                                             z[:, et * 128:(et + 1) * 128],
                                             xwt[:, fo * 512:(fo + 1) * 512],
                                             start=(nk == 0), stop=(nk == NK - 1))
                for et in range(NT):
                    for fo in range(2):
                        nc.vector.tensor_copy(m_sb[:, et, fo * 512:(fo + 1) * 512],
                                              mps[et * 2 + fo])
            with tc.tile_pool(name=f"psD{ci}", bufs=1, space="PSUM") as pD:
                dps = pD.tile([128, NT], F32, name=f"dps{ci}")
                for et in range(NT):
                    nc.tensor.matmul(dps[:, et:et + 1], acc[:, et * 128:(et + 1) * 128],
                                     ones, start=True, stop=True)
                den = sm.tile([128, NT], F32, tag="den", name=f"den{ci}")
                nc.vector.tensor_scalar(den, dps, 1e-16, None, op0=ALU.add)
            rec = sm.tile([128, NT], F32, tag="rec", name=f"rec{ci}")
            nc.vector.reciprocal(rec, den)
            sc = sm.tile([128, NT], F32, tag="sc", name=f"sc{ci}")
            nc.vector.tensor_tensor(sc, rec, rec, op=ALU.mult)
            nc.vector.tensor_tensor(sc, sc, bvk_sb, op=ALU.mult)
            mbf = big.tile([128, NT, F], BF16, tag="mbf", name=f"mbf{ci}")
            for et in range(NT):
                nc.vector.tensor_scalar(mbf[:, et, :], m_sb[:, et, :],
                                        sc[:, et:et + 1], None, op0=ALU.mult)
                nc.sync.dma_start(out=b_m[ci][et * 128:(et + 1) * 128, :],
                                  in_=mbf[:, et, :])
            ag(b_m[ci], g_m[ci])

        def conv_O(ci):
            """out-phase + GraphNorm + leaky -> h1T_sb (bf16)."""
            ae_pk = cst.tile([128, NK], F32, tag="ae_pk", name=f"ae_pk{ci}")
            packed_load(ae_pk, g_ae[ci], "pl2", f"pl_ae{ci}")
            axb_loc = cst.tile([128, S], F32, tag="axb_loc", name=f"axb_loc{ci}")
            nc.gpsimd.dma_start(out=axb_loc, in_=_bcast(b_ax[ci], 0, 1, S))
            hgb_sb = cst.tile([128, KT], F32, tag="hgb", name=f"hgb_sb{ci}")
            nc.sync.dma_start(out=hgb_sb, in_=t_hgb[ci][:])
            hpre = big.tile([128, KT, S], F32, tag="hpre", name=f"hpre{ci}")
            s12 = sm.tile([128, 2 * KT], F32, tag="s12", name=f"s12{ci}")
            with tc.tile_pool(name=f"psO{ci}", bufs=1, space="PSUM") as pO:
                ops_ = [pO.tile([128, 512], F32, tag=f"ops{i}", name=f"ops{ci}_{i}")
                        for i in range(KT)]
                for ek in range(NK):
                    mlh = wk.tile([128, F], BF16, tag="row_fb", name=f"om{ci}_{ek}")
                    nc.sync.dma_start(out=mlh, in_=g_m[ci][ek * 128:(ek + 1) * 128, :])
                    lcnt = wk.tile([128, S], BF16, tag="row_sb", name=f"olc{ci}_{ek}")
                    nc.sync.dma_start(out=lcnt, in_=t_lcn[ek * 128:(ek + 1) * 128, :])
                    zf = wk.tile([128, S], F32, tag="row_s", name=f"ozf{ci}_{ek}")
                    nc.scalar.activation(zf, axb_loc, AF.Prelu,
                                         bias=ae_pk[:, ek:ek + 1], alpha=0.2)
                    nc.vector.tensor_tensor(zf, zf, lcnt, op=ALU.add)
                    zo = wk.tile([128, S], BF16, tag="row_sb2", name=f"oz{ci}_{ek}")
                    nc.scalar.activation(zo, zf, AF.Exp)
                    for ft in range(KT):
                        nc.tensor.matmul(ops_[ft], mlh[:, ft * 128:(ft + 1) * 128], zo,
                                         start=(ek == 0), stop=(ek == NK - 1))
                for ft in range(KT):
                    nc.vector.tensor_tensor(hpre[:, ft, :], ops_[ft], dbc, op=ALU.mult)
                    nc.vector.tensor_scalar(hpre[:, ft, :], hpre[:, ft, :],
                                            hgb_sb[:, ft:ft + 1], None, op0=ALU.add)
                    nc.vector.reduce_sum(s12[:, ft:ft + 1], hpre[:, ft, :], axis=AX)
                    sq = wk.tile([128, S], F32, tag="row_s", name=f"sq{ci}_{ft}")
                    nc.vector.tensor_tensor(sq, hpre[:, ft, :], hpre[:, ft, :], op=ALU.mult)
                    nc.vector.reduce_sum(s12[:, KT + ft:KT + ft + 1], sq, axis=AX)
            nc.sync.dma_start(out=b_gns[ci][:], in_=s12)
            ar(b_gns[ci], g_gns[ci])
            gs = sm.tile([128, 2 * KT], F32, tag="gs", name=f"gs{ci}")
            nc.sync.dma_start(out=gs, in_=g_gns[ci][:])
            gnp = cst.tile([128, 3 * KT], F32, tag="gnp", name=f"gnp{ci}")
            nc.sync.dma_start(out=gnp, in_=t_gn[ci][:])
            mean = sm.tile([128, KT], F32, tag="mean", name=f"mean{ci}")
            nc.vector.tensor_scalar(mean, gs[:, 0:KT], 1.0 / N, None, op0=ALU.mult)
            means = sm.tile([128, KT], F32, tag="means", name=f"means{ci}")
            nc.vector.tensor_tensor(means, mean, gnp[:, 2 * KT:3 * KT], op=ALU.mult)
            var = sm.tile([128, KT], F32, tag="var", name=f"var{ci}")
            nc.vector.tensor_scalar(var, gs[:, KT:2 * KT], 1.0 / N, None, op0=ALU.mult)
            tmpv = sm.tile([128, KT], F32, tag="tmpv", name=f"tmpv{ci}")
            nc.vector.tensor_tensor(tmpv, means, mean, op=ALU.mult)
            nc.vector.tensor_scalar(tmpv, tmpv, 2.0, None, op0=ALU.mult)
            nc.vector.tensor_tensor(var, var, tmpv, op=ALU.subtract)
            nc.vector.tensor_tensor(tmpv, means, means, op=ALU.mult)
            nc.vector.tensor_tensor(var, var, tmpv, op=ALU.add)
            rstd = sm.tile([128, KT], F32, tag="rstd", name=f"rstd{ci}")
            nc.scalar.activation(rstd, var, AF.Sqrt, bias=epsc)
            nc.vector.reciprocal(rstd, rstd)
            gsc = sm.tile([128, KT], F32, tag="gsc", name=f"gsc{ci}")
            nc.vector.tensor_tensor(gsc, gnp[:, 0:KT], rstd, op=ALU.mult)
            gsh = sm.tile([128, KT], F32, tag="gsh", name=f"gsh{ci}")
            nc.vector.tensor_tensor(gsh, means, gsc, op=ALU.mult)
            nc.vector.tensor_tensor(gsh, gnp[:, KT:2 * KT], gsh, op=ALU.subtract)
            for ft in range(KT):
                nc.scalar.activation(h1T_sb[:, ft, :], hpre[:, ft, :], AF.Lrelu,
                                     bias=gsh[:, ft:ft + 1], scale=gsc[:, ft:ft + 1])

        def fc(ci):
            """fc outputs: oT_sb[ci] (T-layout bf16) + node-major bf16 AG."""
            fcb_sb = cst.tile([128, NT], F32, tag="fcb", name=f"fcb_sb{ci}")
            nc.sync.dma_start(out=fcb_sb, in_=t_fcb[ci][:])
            fcbb = cst.tile([128, HID], F32, tag="fcbb", name=f"fcbb{ci}")
            nc.gpsimd.dma_start(out=fcbb, in_=_bcast(t_fcbr[ci], 0, 1, HID))
            with tc.tile_pool(name=f"psF{ci}", bufs=1, space="PSUM") as pF:
                pf1 = [pF.tile([128, S], F32, tag=f"pf1_{i}", name=f"pf1{ci}_{i}")
                       for i in range(NT)]
                pf2 = [pF.tile([128, HID], F32, tag=f"pf2_{i}", name=f"pf2{ci}_{i}")
                       for i in range(NT)]
                for kt in range(KT):
                    fcr = wk.tile([128, HID], BF16, tag="row_hb", name=f"fcr{ci}_{kt}")
                    nc.sync.dma_start(out=fcr, in_=t_fct[ci][kt * 128:(kt + 1) * 128, :])
                    for hot in range(NT):
                        nc.tensor.matmul(pf1[hot], fcr[:, hot * 128:(hot + 1) * 128],
                                         h1T_sb[:, kt, :],
                                         start=(kt == 0), stop=(kt == KT - 1))
                    for nt in range(NT):
                        nc.tensor.matmul(pf2[nt], h1T_sb[:, kt, nt * 128:(nt + 1) * 128],
                                         fcr, start=(kt == 0), stop=(kt == KT - 1))
                for hot in range(NT):
                    nc.scalar.activation(oT_sb[ci][:, hot, :], pf1[hot], AF.Lrelu,
                                         bias=fcb_sb[:, hot:hot + 1])
                for nt in range(NT):
                    tmpo = wk.tile([128, HID], F32, tag="row_h", name=f"ot{ci}_{nt}")
                    nc.vector.tensor_tensor(tmpo, pf2[nt], fcbb, op=ALU.add)
                    onm = wk.tile([128, HID], BF16, tag="row_hb", name=f"onm{ci}_{nt}")
                    nc.scalar.activation(onm, tmpo, AF.Lrelu)
                    nc.sync.dma_start(out=b_o[ci][nt * 128:(nt + 1) * 128, :], in_=onm)
            ag(b_o[ci], g_o[ci])

        def att_round(rnd):
            with tc.tile_pool(name=f"psQ{rnd}", bufs=1, space="PSUM") as pQ:
                qps = [pQ.tile([128, 512], F32, tag=f"qps{i}", name=f"qps{rnd}_{i}")
                       for i in range(8)]
                for nk in range(NK):
                    rhs = wk.tile([128, F], BF16, tag="row_fb", name=f"qr{rnd}_{nk}")
                    if rnd == 0:
                        nc.sync.dma_start(out=rhs, in_=t_xbf[nk * 128:(nk + 1) * 128, :])
                    else:
                        nc.sync.dma_start(out=rhs[:, 0:HID],
                                          in_=g_o[0][nk * 128:(nk + 1) * 128, :])
                        nc.sync.dma_start(out=rhs[:, HID:F],
                                          in_=g_o[1][nk * 128:(nk + 1) * 128, :])
                    lhs = wk.tile([128, S], BF16, tag="row_sb", name=f"ql{rnd}_{nk}")
                    nc.sync.dma_start(out=lhs, in_=t_a1wt[nk * 128:(nk + 1) * 128, :])
                    for jt in range(NT):
                        for cb in range(2):
                            nc.tensor.matmul(qps[jt * 2 + cb],
                                             lhs[:, jt * 128:(jt + 1) * 128],
                                             rhs[:, cb * 512:(cb + 1) * 512],
                                             start=(nk == 0), stop=(nk == NK - 1))
                for jt in range(NT):
                    for cb in range(2):
                        zq = wk.tile([128, 512], F32, tag="row_s", name=f"zq{rnd}_{jt}_{cb}")
                        nc.scalar.activation(zq, qps[jt * 2 + cb], AF.Relu,
                                             bias=a1b_sb[:, jt:jt + 1])
                        nc.vector.tensor_scalar(zq, zq, a2w_sb[:, jt:jt + 1],
                                                None, op0=ALU.mult)
                        if jt == 0:
                            nc.vector.tensor_copy(s_acc[:, rnd * 2 + cb, :], zq)
                        else:
                            nc.vector.tensor_tensor(s_acc[:, rnd * 2 + cb, :],
                                                    s_acc[:, rnd * 2 + cb, :], zq,
                                                    op=ALU.add)

        # ======== phase schedule (AGs overlapped with independent work) ======
        conv_A(0, xT_sb)      # ... AG xw1/ax1/ae1 (hidden by ew matmuls)
        conv_M(0)             # ... AG m1
        att_round(0)          # independent; hides AG m1
        conv_O(0)             # h1
        conv_A(1, h1T_sb)     # ... AG xw2
        fc(0)                 # hides AG xw2; AG o1 at end
        conv_M(1)             # ... AG m2
        conv_O(1)             # h2
        fc(1)                 # AG o2
        att_round(1)

        # ---- s vector + logits ----
        s_sb = sm.tile([128, 16], F32)
        with tc.tile_pool(name="psS", bufs=1, space="PSUM") as pS:
            sps = pS.tile([128, 16], F32)
            sflat = s_acc.rearrange("p a c -> p (a c)")
            for ct in range(16):
                nc.tensor.matmul(sps[:, ct:ct + 1], sflat[:, ct * 128:(ct + 1) * 128],
                                 ones, start=True, stop=True)
            nc.vector.tensor_copy(s_sb, sps)
        nc.sync.dma_start(out=b_s[:], in_=s_sb)
        ar(b_s, g_s)
        ss = sm.tile([128, 16], F32)
        nc.sync.dma_start(out=ss, in_=g_s[:])
        a2bb = cst.tile([128, 1], F32)
        nc.gpsimd.dma_start(out=a2bb, in_=_bcast(t_a2b, 0, 1, 1))
        nc.vector.tensor_scalar(ss, ss, a2bb, None, op0=ALU.add)
        nc.scalar.activation(ss, ss, AF.Sigmoid)
        srow = sm.tile([128, 1], F32)
        nc.vector.reduce_sum(srow, ss, axis=AX)
        with tc.tile_pool(name="psSM", bufs=1, space="PSUM") as pSM:
            smps = pSM.tile([1, 1], F32)
            nc.tensor.matmul(smps, srow, ones, start=True, stop=True)
            smt = sm.tile([1, 1], F32)
            nc.vector.tensor_copy(smt, smps)
        nc.sync.dma_start(out=b_sm[:], in_=smt)
        smb = sm.tile([128, 1], F32)
        nc.gpsimd.dma_start(out=smb, in_=_bcast(b_sm, 0, 1, 1))
        nc.vector.tensor_scalar(smb, smb, 1.0 / (2 * F), None, op0=ALU.mult)
        nc.vector.tensor_scalar(ss, ss, smb, None, op0=ALU.subtract)

        clsw_sb = cst.tile([128, 16, 4], F32)
        nc.sync.dma_start(out=clsw_sb, in_=t_clsw.ap().rearrange("(ct p) o -> p ct o", p=128))
        clswb = cst.tile([128, 16, 4], BF16)
        for ct in range(16):
            nc.vector.tensor_scalar(clswb[:, ct, :], clsw_sb[:, ct, :],
                                    ss[:, ct:ct + 1], None, op0=ALU.mult)
        clsbb = cst.tile([128, 4], F32)
        nc.gpsimd.dma_start(out=clsbb, in_=_bcast(t_clsb, 0, 1, 4))
        lg_sb = sm.tile([128, NT, 4], F32)
        with tc.tile_pool(name="psL", bufs=2, space="PSUM") as pL:
            for nt in range(NT):
                ps = pL.tile([128, 4], F32, tag="psl", name=f"psl{nt}")
                for ct in range(16):
                    if ct < 8:
                        lhsT = xT_sb[:, ct, nt * 128:(nt + 1) * 128]
                    elif ct < 12:
                        lhsT = o1T_sb[:, ct - 8, nt * 128:(nt + 1) * 128]
                    else:
                        lhsT = o2T_sb[:, ct - 12, nt * 128:(nt + 1) * 128]
                    nc.tensor.matmul(ps, lhsT, clswb[:, ct, :],
                                     start=(ct == 0), stop=(ct == 15))
                nc.vector.tensor_tensor(lg_sb[:, nt, :], ps, clsbb, op=ALU.add)
        nc.sync.dma_start(out=t_y.ap().rearrange("(nt p) o -> p nt o", p=128), in_=lg_sb)

        for c in reversed(ctxs):
            c.__exit__(None, None, None)

    nc.compile()
    return nc


# ====================== host side ======================

def _preprocess(inputs):
    x = np.ascontiguousarray(np.asarray(inputs["x"], np.float32))
    ea = np.ascontiguousarray(np.asarray(inputs["edge_attr"], np.float32))
    ei = np.asarray(inputs["edge_index"])
    row = np.asarray(ei[0], np.int64)
    col = np.asarray(ei[1], np.int64)

    C = np.zeros((E, N), np.float32)
    np.add.at(C, (col, row), 1.0)
    LC = np.where(C > 0, np.log(np.maximum(C, 1e-30)), -60.0).astype(np.float32)
    deg_n = np.bincount(row, minlength=N).astype(np.float32)
    deg_e = np.bincount(col, minlength=E).astype(np.float32)
    D = np.where(deg_n > 0, 1.0 / np.maximum(deg_n, 1), 0.0).astype(np.float32)
    B = np.where(deg_e > 0, 1.0 / np.maximum(deg_e, 1), 0.0).astype(np.float32)

    bf = ml_dtypes.bfloat16
    LCT_bf = LC.T.astype(bf)           # [N, E]
    LC_bf = LC.astype(bf)              # [E, N]
    xbf = x.astype(bf)
    a1w = np.asarray(inputs["att1_W"], np.float32)

    def pack_pp(v, nt):  # [nt*128] -> [128, nt]
        return np.ascontiguousarray(v.reshape(nt, 128).T.astype(np.float32))

    f32 = np.float32
    com = {
        "xbf": xbf,
        "w1t": np.ascontiguousarray(np.asarray(inputs["hg1_W"], f32).T.astype(bf)),
        "w2t": np.ascontiguousarray(np.asarray(inputs["hg2_W"], f32).T.astype(bf)),
        "fc1t": np.ascontiguousarray(np.asarray(inputs["fc1_W"], f32).T.astype(bf)),
        "fc2t": np.ascontiguousarray(np.asarray(inputs["fc2_W"], f32).T.astype(bf)),
        "attx1": np.asarray(inputs["hg1_att"], f32)[:F].reshape(1, F),
        "atte1": np.asarray(inputs["hg1_att"], f32)[F:].reshape(1, F),
        "attx2": np.asarray(inputs["hg2_att"], f32)[:F].reshape(1, F),
        "atte2": np.asarray(inputs["hg2_att"], f32)[F:].reshape(1, F),
        "hgb1": pack_pp(np.asarray(inputs["hg1_b"], f32), KT),
        "hgb2": pack_pp(np.asarray(inputs["hg2_b"], f32), KT),
        "gn1": np.concatenate([pack_pp(np.asarray(inputs[k], f32), KT)
                               for k in ("gn1_w", "gn1_b", "gn1_ms")], axis=1),
        "gn2": np.concatenate([pack_pp(np.asarray(inputs[k], f32), KT)
                               for k in ("gn2_w", "gn2_b", "gn2_ms")], axis=1),
        "fcb1": pack_pp(np.asarray(inputs["fc1_b"], f32), NT),
        "fcb2": pack_pp(np.asarray(inputs["fc2_b"], f32), NT),
        "fcb1r": np.asarray(inputs["fc1_b"], f32).reshape(1, HID),
        "fcb2r": np.asarray(inputs["fc2_b"], f32).reshape(1, HID),
        "a2b": np.asarray(inputs["att2_b"], f32).reshape(1, 1),
        "clsw": np.ascontiguousarray(np.asarray(inputs["cls_W"], f32).T),
        "clsb": np.asarray(inputs["cls_b"], f32).reshape(1, 4),
    }
    att1_b = np.asarray(inputs["att1_b"], f32)
    att2_w = np.asarray(inputs["att2_W"], f32)[0]

    in_maps = []
    for k in range(NCORES):
        sl = slice(k * S, (k + 1) * S)
        m = dict(com)
        m["xT_k"] = np.ascontiguousarray(x[sl].T.astype(bf))
        m["eaT_k"] = np.ascontiguousarray(ea[sl].T.astype(bf))
        m["lct_k"] = np.ascontiguousarray(LCT_bf[:, sl])
        m["lcn_k"] = np.ascontiguousarray(LC_bf[:, sl])
        m["a1wt_k"] = np.ascontiguousarray(a1w[sl].T.astype(bf))
        m["dvec_k"] = D[sl].reshape(1, S).copy()
        m["bvec_k"] = pack_pp(B[sl], NT)
        m["a1b_k"] = pack_pp(att1_b[sl], NT)
        m["a2w_k"] = pack_pp(att2_w[sl], NT)
        in_maps.append(m)
    return in_maps


def kernel(**inputs) -> np.ndarray:
    if "nc" not in _CACHE:
        _CACHE["nc"] = build_program()
    nc = _CACHE["nc"]
    in_maps = _preprocess(inputs)
    last_err = None
    for _ in range(3):
        try:
            res = run_bass_kernel_spmd(nc, in_maps, list(range(NCORES))).results
            return np.concatenate([res[k]["y"] for k in range(NCORES)], axis=0)
        except Exception as e:  # flaky NRT_EXEC_UNIT_UNRECOVERABLE retries
            last_err = e
    raise last_err
